# revision 17
# baseline (speedup 1.0000x reference)
"""Trainium2 Bass kernel for an AxialAttentionLayer-style module.

Math: for each batch b,
    scores = q @ k'          where k'[e,j] = keys[e,j] + sum_d keys[j,d]
    A      = softmax(scores, axis=-1)
    out    = A @ values
(the reference's rank-1 additive score s1 folds into the matmul because
 s1[l,j] = (sum_e q[l,e]) * ksum[j] = sum_e q[l,e]*ksum[j]).

Sharding: data-parallel over batch B=32 across 8 cores (4 batches/core).
Device layout per core, per 512-row block of L:
    MM1   (PE, fp32):   scores(l,s) = qT_chunk.T @ k'   (4x 128-tiles)
    max   (DVE):        fused 3D reduce_max(negate) -> -rowmax (128,4)
    exp   (ACT):        P = exp(scores - rowmax), per-tile bias
    rowsum(DVE):        fused 3D reduce_sum over P -> staged per batch
    T     (PE):         P chunks transposed via PE transpose -> PSUM
    copy  (ACT):        PT PSUM -> SBUF (rounded to fp32r)
    MM2   (PE, fp32r):  outT(d,l) = v.T-free matmul with v stationary
    copy  (DVE):        outT PSUM -> SBUF
Host: pre-transposes q -> qT, builds k', divides by rowsum, transposes back.
"""

import numpy as np

B, L, S = 32, 8192, 128
N_CORES = 8
B_LOC = B // N_CORES  # 4
LBLK = 512            # l-rows per block
NT = LBLK // 128      # 128-tiles per block
NBLK = L // LBLK      # blocks per batch

_RUNNER_CACHE = {}

# tunables (overridable before building)
CFG = dict(
    qt_bufs=4, p_bufs=3, pt_bufs=3, nm_bufs=4, oc_bufs=4, rs_bufs=2,
    sc_bufs=4, ptps_bufs=2, o_bufs=2,
    out_copy_engine="dve",   # "act" | "dve" | "alt"
    pt_copy_engine="act",    # "act" | "dve" | "alt"
    store_engine="sp",       # "pool" | "act" | "dve" | "sp"
    rowsum_mode="dve",       # "dve" | "pool" | "pe"
    mm1_dtype="f32",         # "f32" | "f32r" | "bf16x2"
    out_dtype="f32",         # "f32" | "bf16"
    bias_mode="act",         # "act" (per-tile exp bias) | "pe" (K=4 accum matmul)
    nm_copy_engine="act",    # "dve" | "act"
    scs_copy_engine="dve",   # "dve" | "act"
    dataflow="lsoft",        # "lsoft" (v1: softmax in (l,s)) | "t" (transposed)
    rsps_bufs=1,
    max_out_dtype="f32r",    # partition_all_reduce out dtype in "t" flow
)


def _build_nc(repeat=1, cfg=None):
    cfg = {**CFG, **(cfg or {})}
    import concourse.bacc as bacc
    import concourse.mybir as mybir
    import concourse.tile as tile
    from concourse.bass import ts
    from concourse.masks import make_identity

    f32 = mybir.dt.float32
    f32r = mybir.dt.float32r

    nc = bacc.Bacc("TRN2", target_bir_lowering=False, debug=False)
    bf16 = mybir.dt.bfloat16
    if cfg["mm1_dtype"] == "bf16x2":
        qT_d = nc.dram_tensor("qT2", (B_LOC, S, 2, L), bf16, kind="ExternalInput")
        kph_d = nc.dram_tensor("kph", (B_LOC, S, S), bf16, kind="ExternalInput")
        kpl_d = nc.dram_tensor("kpl", (B_LOC, S, S), bf16, kind="ExternalInput")
    else:
        mm1_dt_glob = f32 if cfg["mm1_dtype"] == "f32" else f32r
        qT_d = nc.dram_tensor("qT", (B_LOC, S, L), mm1_dt_glob, kind="ExternalInput")
    kp_d = nc.dram_tensor("kp", (B_LOC, S, S), f32, kind="ExternalInput")
    v_d = nc.dram_tensor("v", (B_LOC, S, S), f32, kind="ExternalInput")
    ind_d = None
    if cfg["bias_mode"] == "pe":
        ind_d = nc.dram_tensor("ind", (NT, LBLK), f32r, kind="ExternalInput")
    sel16_d = None
    if cfg["dataflow"] == "t":
        sel16_d = nc.dram_tensor("sel16", (S, NBLK * NBLK), f32r,
                                 kind="ExternalInput")
    out_dt = f32 if cfg["out_dtype"] == "f32" else mybir.dt.bfloat16
    outT_d = nc.dram_tensor("outT", (B_LOC, S, L), out_dt, kind="ExternalOutput")
    if cfg["rowsum_mode"] == "dve" and cfg["dataflow"] == "lsoft":
        rs_d = nc.dram_tensor("rs", (B_LOC, S, L // S), f32, kind="ExternalOutput")
    elif cfg["dataflow"] == "t":
        rs_d = nc.dram_tensor("rs", (B_LOC, NBLK, LBLK), f32, kind="ExternalOutput")
    else:
        rs_d = nc.dram_tensor("rs", (B_LOC, L), f32, kind="ExternalOutput")

    from concourse import bass_isa
    Exp = mybir.ActivationFunctionType.Exp
    AX = mybir.AxisListType.X
    MAX = mybir.AluOpType.max
    ADD = mybir.AluOpType.add

    with tile.TileContext(nc) as tc:
        with (
            tc.tile_pool(name="const", bufs=1) as constp,
            tc.tile_pool(name="qt", bufs=cfg["qt_bufs"]) as qtp,
            tc.tile_pool(name="p", bufs=cfg["p_bufs"]) as pp,
            tc.tile_pool(name="pt", bufs=cfg["pt_bufs"]) as ptp,
            tc.tile_pool(name="nm", bufs=cfg["nm_bufs"]) as nmp,
            tc.tile_pool(name="rss", bufs=cfg["rs_bufs"]) as rsp,
            tc.tile_pool(name="oc", bufs=cfg["oc_bufs"]) as ocp,
            tc.tile_pool(name="scps", bufs=cfg["sc_bufs"], space="PSUM") as scps,
            tc.tile_pool(name="ptps", bufs=cfg["ptps_bufs"], space="PSUM") as ptps,
            tc.tile_pool(name="ops", bufs=cfg["o_bufs"], space="PSUM") as ops,
            tc.tile_pool(name="rsps", bufs=cfg["rsps_bufs"], space="PSUM") as rspsp,
            tc.tile_pool(name="auxps", bufs=1, space="PSUM") as auxps,
        ):
            ident = constp.tile([128, 128], f32, tag="ident")
            make_identity(nc, ident[:])
            kp_sb = constp.tile([128, B_LOC * 128], f32, tag="kp")
            v_sb = constp.tile([128, B_LOC * 128], f32, tag="v")
            v_r = constp.tile([128, B_LOC * 128], f32r, tag="vr")
            ind_r = None
            if cfg["bias_mode"] == "pe":
                ind_r = constp.tile([NT, LBLK], f32r, tag="ind")
                nc.sync.dma_start(ind_r[:], ind_d[:])
            ones_r = None
            if cfg["rowsum_mode"] == "pe" or cfg["dataflow"] == "t":
                ones_f = constp.tile([128, 1], f32, tag="ones_f")
                ones_r = constp.tile([128, 1], f32r, tag="ones")
                nc.gpsimd.memset(ones_f[:], 1.0)
                nc.vector.tensor_copy(ones_r[:], ones_f[:])
            neg_inv_r = None
            sel16_r = None
            if cfg["dataflow"] == "t":
                neg_inv_f = constp.tile([128, 128], f32, tag="ninv_f")
                neg_inv_r = constp.tile([128, 128], f32r, tag="ninv")
                nc.gpsimd.memset(neg_inv_f[:], -1.0 / 128.0)
                nc.vector.tensor_copy(neg_inv_r[:], neg_inv_f[:])
                sel16_r = constp.tile([128, NBLK * NBLK], f32r, tag="sel16")
                nc.sync.dma_start(sel16_r[:], sel16_d[:])
            for b in range(B_LOC):
                nc.sync.dma_start(kp_sb[:, ts(b, 128)], kp_d[b])
                nc.sync.dma_start(v_sb[:, ts(b, 128)], v_d[b])
            nc.vector.tensor_copy(v_r[:], v_sb[:])
            kp_r = None
            if cfg["mm1_dtype"] == "f32r":
                kp_r = constp.tile([128, B_LOC * 128], f32r, tag="kpr")
                nc.vector.tensor_copy(kp_r[:], kp_sb[:])
            kph_sb = kpl_sb = None
            if cfg["mm1_dtype"] == "bf16x2":
                bf16_ = mybir.dt.bfloat16
                kph_sb = constp.tile([128, B_LOC * 128], bf16_, tag="kph")
                kpl_sb = constp.tile([128, B_LOC * 128], bf16_, tag="kpl")
                for b in range(B_LOC):
                    nc.sync.dma_start(kph_sb[:, ts(b, 128)], kph_d[b])
                    nc.sync.dma_start(kpl_sb[:, ts(b, 128)], kpl_d[b])

            def t_block(b, blk, rs_stage, rs_ps_holder):
                l0 = blk * LBLK
                sc = scps.tile([128, LBLK], f32, tag="sc")
                if cfg["mm1_dtype"] == "bf16x2":
                    bf16_ = mybir.dt.bfloat16
                    qt2 = qtp.tile([128, 2 * LBLK], bf16_, tag="qt")
                    nc.sync.dma_start(
                        qt2[:].rearrange("p (h l) -> p h l", h=2),
                        qT_d[b, :, :, l0:l0 + LBLK])
                    qh = qt2[:, 0:LBLK]
                    ql = qt2[:, LBLK:2 * LBLK]
                    nc.tensor.matmul(sc[:], kph_sb[:, ts(b, 128)], qh,
                                     start=True, stop=False)
                    nc.tensor.matmul(sc[:], kpl_sb[:, ts(b, 128)], qh,
                                     start=False, stop=False)
                    nc.tensor.matmul(sc[:], kph_sb[:, ts(b, 128)], ql,
                                     start=False, stop=False)
                else:
                    mm1_dt = f32 if cfg["mm1_dtype"] == "f32" else f32r
                    kp_use = kp_sb if cfg["mm1_dtype"] == "f32" else kp_r
                    qt = qtp.tile([128, LBLK], mm1_dt, tag="qt")
                    nc.sync.dma_start(qt[:], qT_d[b, :, l0:l0 + LBLK])
                    nc.tensor.matmul(sc[:], kp_use[:, ts(b, 128)], qt[:],
                                     start=True, stop=False)
                scs = pp.tile([128, LBLK], f32, tag="scs")
                if cfg["scs_copy_engine"] == "dve":
                    nc.vector.tensor_copy(scs[:], sc[:])
                else:
                    nc.scalar.copy(scs[:], sc[:])
                mx_dt = f32r if cfg["max_out_dtype"] == "f32r" else f32
                mxr = ptp.tile([128, LBLK], mx_dt, tag="mxr")
                nc.gpsimd.partition_all_reduce(
                    mxr[:], scs[:], 128, bass_isa.ReduceOp.max)
                nc.tensor.matmul(sc[:], neg_inv_r[:], mxr[:],
                                 start=False, stop=True)
                pt = ptp.tile([128, LBLK], f32r, tag="pt")
                nc.scalar.activation(pt[:], sc[:], Exp, bias=0.0, scale=1.0)
                if blk == 0:
                    rs_ps_new = rspsp.tile([NBLK, LBLK], f32, tag="rsps")
                    rs_ps_holder[0] = rs_ps_new
                rs_ps = rs_ps_holder[0]
                nc.tensor.matmul(rs_ps[:], sel16_r[:, blk * NBLK:(blk + 1) * NBLK],
                                 pt[:], start=(blk == 0), stop=(blk == NBLK - 1))
                if blk == NBLK - 1:
                    rsx = rsp.tile([NBLK, LBLK], f32, tag="rsx")
                    nc.vector.tensor_copy(rsx[:], rs_ps[:])
                    nc.sync.dma_start(rs_d[b], rsx[:])
                op_t = ops.tile([128, LBLK], f32, tag="op")
                nc.tensor.matmul(op_t[:], v_r[:, ts(b, 128)], pt[:],
                                 start=True, stop=True)
                oc = ocp.tile([128, LBLK], out_dt, tag="oc")
                oce = cfg["out_copy_engine"]
                if oce in ("alt", "act") or oce.startswith("mix"):
                    nc.scalar.copy(oc[:], op_t[:])
                else:
                    nc.vector.tensor_copy(oc[:], op_t[:])
                st = {"pool": nc.gpsimd, "act": nc.scalar,
                      "dve": nc.vector, "sp": nc.sync}[cfg["store_engine"]]
                st.dma_start(outT_d[b, :, l0:l0 + LBLK], oc[:])

            def t_body(_iv=None):
                for b in range(B_LOC):
                    holder = [None]
                    for blk in range(NBLK):
                        t_block(b, blk, None, holder)

            def body(_iv=None):
                if cfg["dataflow"] == "t":
                    return t_body(_iv)
                for b in range(B_LOC):
                    mode = cfg["rowsum_mode"]
                    if mode == "dve":
                        rs_stage = rsp.tile([128, L // S], f32, tag="rss")
                    elif mode == "pool":
                        rs_stage = rsp.tile([128, L], f32, tag="rss")
                    else:
                        rs_stage = rsp.tile([16, LBLK], f32, tag="rss")
                    for blk in range(NBLK):
                        l0 = blk * LBLK
                        mm1_dt = f32 if cfg["mm1_dtype"] == "f32" else f32r
                        qt = qtp.tile([128, LBLK], mm1_dt, tag="qt")
                        nc.sync.dma_start(qt[:], qT_d[b, :, l0:l0 + LBLK])
                        sc = scps.tile([128, LBLK], f32, tag="sc")
                        for ti in range(NT):
                            nc.tensor.matmul(
                                sc[:, ts(ti, 128)], qt[:, ts(ti, 128)],
                                (kp_sb if cfg["mm1_dtype"] == "f32" else kp_r)[:, ts(b, 128)],
                                start=True,
                                stop=(cfg["bias_mode"] == "act"),
                                skip_group_check=(cfg["bias_mode"] == "pe"))
                        nm = nmp.tile([128, NT], f32, tag="nm")
                        nc.vector.tensor_reduce(
                            nm[:], sc[:].rearrange("p (t s) -> p t s", t=NT),
                            axis=AX, op=MAX, negate=True)
                        p = pp.tile([128, LBLK], f32, tag="p")
                        if cfg["bias_mode"] == "act":
                            for ti in range(NT):
                                nc.scalar.activation(
                                    p[:, ts(ti, 128)], sc[:, ts(ti, 128)], Exp,
                                    bias=nm[:, ti:ti + 1], scale=1.0)
                        else:
                            nmt_ps = auxps.tile([NT, 128], f32, tag="nmt")
                            nc.tensor.transpose(nmt_ps[:], nm[:], ident[:])
                            nmt = nmp.tile([NT, 128], f32r, tag="nmtr")
                            if cfg["nm_copy_engine"] == "dve":
                                nc.vector.tensor_copy(nmt[:], nmt_ps[:])
                            else:
                                nc.scalar.copy(nmt[:], nmt_ps[:])
                            nc.tensor.matmul(sc[:], nmt[:], ind_r[:],
                                             start=False, stop=True,
                                             skip_group_check=True)
                            nc.scalar.activation(p[:], sc[:], Exp,
                                                 bias=0.0, scale=1.0)
                        if cfg["rowsum_mode"] == "dve":
                            nc.vector.tensor_reduce(
                                rs_stage[:, blk * NT:(blk + 1) * NT],
                                p[:].rearrange("p (t s) -> p t s", t=NT),
                                axis=AX, op=ADD)
                        ptps_t = ptps.tile([128, LBLK], f32, tag="ptps")
                        for ti in range(NT):
                            nc.tensor.transpose(
                                ptps_t[:, ts(ti, 128)], p[:, ts(ti, 128)],
                                ident[:])
                        pt = ptp.tile([128, LBLK], f32r, tag="pt")
                        pce = cfg["pt_copy_engine"]
                        if pce == "alt":
                            pce = "dve" if blk % 2 == 0 else "act"
                        elif pce.startswith("mix"):
                            n, m = pce[3:].split("of")
                            pce = "dve" if blk % int(m) < int(n) else "act"
                        if pce == "dve":
                            nc.vector.tensor_copy(pt[:], ptps_t[:])
                        else:
                            nc.scalar.copy(pt[:], ptps_t[:])
                        if cfg["rowsum_mode"] == "pool":
                            nc.gpsimd.partition_all_reduce(
                                rs_stage[:, blk * LBLK:(blk + 1) * LBLK],
                                pt[:], 128, bass_isa.ReduceOp.add)
                        elif cfg["rowsum_mode"] == "pe":
                            if blk % 4 == 0:
                                rs_ps = rspsp.tile([128, LBLK], f32, tag="rsps")
                            j = blk % 4
                            nc.tensor.matmul(
                                rs_ps[32 * j:32 * j + 1, :], ones_r[:], pt[:],
                                start=True, stop=True,
                                tile_position=(0, 32 * j))
                            if j == 3:
                                nc.vector.tensor_copy(
                                    rs_stage[(blk - 3) // 4 * 4:(blk - 3) // 4 * 4 + 4, :].rearrange("a b -> a b"),
                                    rs_ps[:].rearrange("(a c) b -> a c b", c=32)[:, 0:1, :].rearrange("a c b -> (a c) b"))
                        op_t = ops.tile([128, LBLK], f32, tag="op")
                        nc.tensor.matmul(
                            op_t[:], v_r[:, ts(b, 128)], pt[:],
                            start=True, stop=True)
                        oc = ocp.tile([128, LBLK], out_dt, tag="oc")
                        oce = cfg["out_copy_engine"]
                        if oce == "alt":
                            oce = "act" if blk % 2 == 0 else "dve"
                        elif oce.startswith("mix"):
                            n, m = oce[3:].split("of")
                            oce = "dve" if blk % int(m) < int(n) else "act"
                        if oce == "act":
                            nc.scalar.copy(oc[:], op_t[:])
                        else:
                            nc.vector.tensor_copy(oc[:], op_t[:])
                        st = {"pool": nc.gpsimd, "act": nc.scalar,
                              "dve": nc.vector, "sp": nc.sync}[cfg["store_engine"]]
                        st.dma_start(outT_d[b, :, l0:l0 + LBLK], oc[:])
                    if cfg["rowsum_mode"] == "dve":
                        nc.gpsimd.dma_start(rs_d[b], rs_stage[:])
                    elif cfg["rowsum_mode"] == "pool":
                        nc.sync.dma_start(rs_d[b], rs_stage[0:1, :].rearrange("a b -> (a b)"))
                    else:
                        nc.sync.dma_start(rs_d[b], rs_stage[:].rearrange("a b -> (a b)"))

            if repeat == 1:
                body()
            else:
                with tc.For_i(0, repeat, 1) as _i:
                    body(_i)

    nc.compile()
    return nc


def _make_runner(repeat=1, cfg=None):
    """Compile (once) and return fn(in_maps) -> list[dict] per core."""
    key = (repeat, tuple(sorted((cfg or {}).items())))
    if key in _RUNNER_CACHE:
        return _RUNNER_CACHE[key]

    import jax
    import concourse.mybir as mybir
    from concourse import bass2jax
    from concourse.bass2jax import _bass_exec_p, partition_id_tensor
    from jax.sharding import Mesh, NamedSharding, PartitionSpec
    from jax.experimental.shard_map import shard_map

    nc = _build_nc(repeat, cfg)
    bass2jax.install_neuronx_cc_hook()

    in_names, out_names, out_avals, zero_shapes = [], [], [], []
    for alloc in nc.m.functions[0].allocations:
        if not isinstance(alloc, mybir.MemoryLocationSet):
            continue
        name = alloc.memorylocations[0].name
        if alloc.kind == "ExternalInput":
            if nc.partition_id_tensor is None or name != nc.partition_id_tensor.name:
                in_names.append(name)
        elif alloc.kind == "ExternalOutput":
            out_names.append(name)
            shape = tuple(alloc.tensor_shape)
            dtype = mybir.dt.np(alloc.dtype)
            out_avals.append(jax.core.ShapedArray(shape, dtype))
            zero_shapes.append((shape, dtype))
    n_params = len(in_names)
    pid_name = nc.partition_id_tensor.name if nc.partition_id_tensor else None
    names_for_bind = in_names + out_names + ([pid_name] if pid_name else [])

    def _body(*args):
        operands = list(args)
        if pid_name:
            operands.append(partition_id_tensor())
        outs = _bass_exec_p.bind(
            *operands,
            out_avals=tuple(out_avals),
            in_names=tuple(names_for_bind),
            out_names=tuple(out_names),
            lowering_input_output_aliases=(),
            sim_require_finite=True,
            sim_require_nnan=True,
            nc=nc,
        )
        return tuple(outs)

    devices = jax.devices()[:N_CORES]
    mesh = Mesh(np.asarray(devices), ("core",))
    nspec = n_params + len(out_names)
    fn = jax.jit(
        shard_map(_body, mesh=mesh,
                  in_specs=(PartitionSpec("core"),) * nspec,
                  out_specs=(PartitionSpec("core"),) * len(out_names),
                  check_rep=False),
        keep_unused=True)
    sharding = NamedSharding(mesh, PartitionSpec("core"))

    def run(in_maps):
        import jax as _jax
        concat_in = [
            np.concatenate([np.asarray(m[name]) for m in in_maps], axis=0)
            for name in in_names
        ]
        zeros = [np.zeros((N_CORES * s[0],) + tuple(s[1:]), d)
                 for (s, d) in zero_shapes]
        dev_in = [_jax.device_put(a, sharding) for a in concat_in + zeros]
        out_arrs = fn(*dev_in)
        _jax.block_until_ready(out_arrs)
        return [
            {name: np.asarray(out_arrs[i]).reshape(
                (N_CORES,) + tuple(out_avals[i].shape))[c]
             for i, name in enumerate(out_names)}
            for c in range(N_CORES)
        ], (fn, dev_in)

    _RUNNER_CACHE[key] = run
    return run


def _prep_inputs(queries, keys, values, cfg=None):
    cfg = {**CFG, **(cfg or {})}
    qT = np.ascontiguousarray(queries.transpose(0, 2, 1))      # (B, E, L)
    kp = keys + keys.sum(axis=2)[:, None, :]                   # k' = k + 1*ksum
    kp = np.ascontiguousarray(kp.astype(np.float32))
    v = np.ascontiguousarray(values.astype(np.float32))
    qT2 = kph = kpl = None
    if cfg["mm1_dtype"] == "bf16x2":
        import ml_dtypes
        bf = ml_dtypes.bfloat16
        qTh = qT.astype(bf)
        qTl = (qT - qTh.astype(np.float32)).astype(bf)
        qT2 = np.ascontiguousarray(np.stack([qTh, qTl], axis=2))
        kph = kp.astype(bf)
        kpl = np.ascontiguousarray((kp - kph.astype(np.float32)).astype(bf))
        kph = np.ascontiguousarray(kph)
    ind = np.zeros((NT, LBLK), np.float32)
    for ti in range(NT):
        ind[ti, ti * 128:(ti + 1) * 128] = 1.0
    sel16 = np.zeros((S, NBLK * NBLK), np.float32)
    for j in range(NBLK):
        sel16[:, j * NBLK + j] = 1.0
    in_maps = []
    for c in range(N_CORES):
        sl = slice(c * B_LOC, (c + 1) * B_LOC)
        m = {"qT": qT[sl], "kp": kp[sl], "v": v[sl], "ind": ind,
             "sel16": sel16}
        if qT2 is not None:
            m.update({"qT2": qT2[sl], "kph": kph[sl], "kpl": kpl[sl]})
        in_maps.append(m)
    return in_maps


def _assemble(results, cfg=None):
    cfg = {**CFG, **(cfg or {})}
    out = np.empty((B, L, S), dtype=np.float32)
    for c in range(N_CORES):
        outT = results[c]["outT"]          # (B_LOC, S, L)  = out^T per batch
        rs = results[c]["rs"]              # (B_LOC, 128, L//128) rowsums
        for b in range(B_LOC):
            if cfg["rowsum_mode"] == "dve" and cfg["dataflow"] == "lsoft":
                rsum = rs[b].T.reshape(L)  # rowsum[l]
            else:
                rsum = rs[b].reshape(L)
            out[c * B_LOC + b] = outT[b].T / rsum[:, None]
    return out.reshape(B, 1, L, S)


def kernel(queries, keys, values):
    run = _make_runner(repeat=1)
    in_maps = _prep_inputs(queries, keys, values)
    results, _ = run(in_maps)
    return _assemble(results)
